# revision 11
# baseline (speedup 1.0000x reference)
"""Boundary-loss Trainium2 kernel (Bass/Tile), SPMD over 8 NeuronCores.

loss = mean(softmax(logits, C) * phi(targets)), phi the signed EDT map.
Per pixel p with target t:  sum_c probs_c*phi_c = (sum_c e_c R_c - e_t*m2)/S_e + 1
with R_c = sqrt(edt2(mask_c)), m2 = min_{c!=t} R_c (= second-smallest R).

Device algorithm (one batch item per core, bf16 maps unless noted):
  * masks F_c = (t != c)*BIG with BIG pad columns; the 1-D L1 row distance
    runs as TWO flattened tensor_tensor_scans (fw + reversed bw) on DVE,
    then one strided row-min.
  * PE transposes logits (fp32) and rho (bf16) blockwise; ACT evicts PSUM
    fused with Exp (E = e^logit) resp. Square (d1 = rho^2). Everything
    downstream stays in transposed space -- no back-transposes.
  * col pass: exact windowed parabolic mins, window K: ACT/DVE prebuild
    TMPA_d = d1 + d^2, DVE runs one merged in-place 2x-mode min chain over
    all 4 classes (2 shifted mins per delta).
  * R = sqrt(D) on ACT; order stats run on R, so m2 needs no extra sqrt and
    the exactness certificate is max(R) <= K+1 (any pixel whose computed D
    is <= (K+1)^2 is provably exact; host retries with K+1 else).
  * tail on raw e_c (softmax never materialized): e_t via [d1==0]
    indicators (DVE 4x tensor_scalar) with mult/add chains on GPSIMD;
    S = sum_c e_c R_c; two fused scalar_tensor_tensor accumulations fold
    the single 1/S_e map into per-partition sums of S/S_e and e_t*m2/S_e;
    the host subtracts them and adds the +1/C term.
"""
from contextlib import ExitStack

import numpy as np

import concourse.bass as bass
import concourse.tile as tile
from concourse import bacc, mybir
from concourse.bass_utils import run_bass_kernel_spmd
from concourse.masks import make_identity

P = 128
C = 4
H = W = 384
KCH = H // P     # 3 row chunks (natural space)
KW = W // P      # 3 col chunks (transposed space)
PAD = 8
WP = W + PAD     # padded row length for the flattened scans
FLAT = C * KCH * WP
N_CORES = 8
BIG = 65536.0
DEFAULT_K = 4    # parabolic window; exact iff max R <= K+1 (certified)

FP32 = mybir.dt.float32
BF16 = mybir.dt.bfloat16
INT32 = mybir.dt.int32
OP = mybir.AluOpType
ACT = mybir.ActivationFunctionType


def _build_nc(K: int) -> bass.Bass:
    nc = bacc.Bacc("TRN2", target_bir_lowering=False, debug=False)
    logits_d = nc.dram_tensor("logits", [C, H, W], FP32, kind="ExternalInput")
    targets_d = nc.dram_tensor("targets", [H, W], INT32, kind="ExternalInput")
    out_d = nc.dram_tensor("out", [P, 4], FP32, kind="ExternalOutput")

    with tile.TileContext(nc) as tc, ExitStack() as ctx:
        pool = ctx.enter_context(tc.tile_pool(name="main", bufs=1))
        psq = ctx.enter_context(tc.tile_pool(name="psq", bufs=2, space="PSUM"))
        psl = ctx.enter_context(tc.tile_pool(name="psl", bufs=2, space="PSUM"))

        # ---- loads ----
        T = pool.tile([P, KCH, W], INT32)
        tr = targets_d[:].rearrange("(k p) w -> p k w", p=P)
        for k in range(KCH):
            nc.sync.dma_start(T[:, k], tr[:, k])
        L = pool.tile([P, C, KCH, W], FP32)
        nc.scalar.dma_start(L[:], logits_d[:].rearrange("c (k p) w -> p c k w", p=P))

        # ---- constants ----
        ONES = pool.tile([P, 1], BF16)
        nc.vector.memset(ONES[:], 1.0)
        IDENT = pool.tile([P, P], BF16)
        make_identity(nc, IDENT[:])
        IDF = pool.tile([P, P], FP32)
        make_identity(nc, IDF[:])
        BIASQ = pool.tile([P, K], FP32)
        for d in range(2, K + 1):
            nc.vector.memset(BIASQ[:, d - 1:d], float(d * d))

        # ---- masks F_c = (t != c)*BIG, with BIG pad columns ----
        F = pool.tile([P, C, KCH, WP], BF16)
        nc.gpsimd.memset(F[:, :, :, W:WP], BIG)
        TFb = pool.tile([P, KCH, W], BF16)
        for k in range(KCH):
            nc.scalar.copy(TFb[:, k], T[:, k])
        for c in range(C):
            nc.vector.tensor_scalar(F[:, c, :, 0:W], TFb[:], float(c), BIG,
                                    op0=OP.not_equal, op1=OP.mult)

        # ---- row pass: flattened L1 scans, then min ----
        FW = pool.tile([P, C, KCH, WP], BF16)
        BW = pool.tile([P, C, KCH, WP], BF16)
        ff = F[:].rearrange("p c k w -> p (c k w)")
        fwf = FW[:].rearrange("p c k w -> p (c k w)")
        bwf = BW[:].rearrange("p c k w -> p (c k w)")
        ONESB = ONES[:, 0:1].broadcast_to([P, FLAT])
        nc.vector.tensor_tensor_scan(fwf, ONESB, ff, BIG,
                                     op0=OP.add, op1=OP.min)
        nc.vector.tensor_tensor_scan(bwf[:, ::-1], ONESB, ff[:, ::-1], BIG,
                                     op0=OP.add, op1=OP.min)
        for h in range(2):
            nc.vector.tensor_tensor(FW[:, 2 * h:2 * h + 2, :, 0:W],
                                    FW[:, 2 * h:2 * h + 2, :, 0:W],
                                    BW[:, 2 * h:2 * h + 2, :, 0:W],
                                    op=OP.min)  # rho

        # ---- PE transposes + fused evictions ----
        # logits first (PE is idle while scans run; E feeds the mid chains),
        # then rho per class (feeds the col pass).
        ET4 = pool.tile([P, C, KW, H], BF16)   # e^logit, transposed
        D1T = pool.tile([P, C, KW, H], BF16)   # rho^2, transposed
        for c in range(C):
            for kw in range(KW):
                pl = psl.tile([P, KCH, P], FP32, tag="psl")
                for kh in range(KCH):
                    nc.tensor.matmul(pl[:, kh, :],
                                     L[:, c, kh, kw * P:(kw + 1) * P],
                                     IDF[:], is_transpose=True)
                nc.scalar.activation(
                    ET4[:, c, kw], pl[:].rearrange("p kh x -> p (kh x)"),
                    ACT.Exp)
        for c in range(C):
            p9 = psq.tile([P, KW, KCH, P], BF16, tag="ps9")
            for kw in range(KW):
                for kh in range(KCH):
                    nc.tensor.matmul(p9[:, kw, kh, :],
                                     FW[:, c, kh, kw * P:(kw + 1) * P],
                                     IDENT[:], is_transpose=True)
            nc.scalar.activation(
                D1T[:, c], p9[:].rearrange("p kw kh x -> p kw (kh x)"),
                ACT.Square)

        # ---- S_e and 1/S_e (fills the DVE gap while rho transposes run;
        # 1/S_e folds into the two final fused accumulations) ----
        SE = pool.tile([P, KW, H], BF16)
        TMP = pool.tile([P, KW, H], BF16)
        nc.gpsimd.tensor_tensor(SE[:], ET4[:, 0], ET4[:, 1], op=OP.add)
        nc.gpsimd.tensor_tensor(TMP[:], ET4[:, 2], ET4[:, 3], op=OP.add)
        nc.gpsimd.tensor_tensor(SE[:], SE[:], TMP[:], op=OP.add)
        RC = pool.tile([P, KW, H], FP32)
        nc.vector.reciprocal(RC[:], SE[:])

        # ---- col pass, split into two independent 2-class chains so the
        # first half's sqrt/stats/products start while the second half runs.
        # TMPA_d = d1 + d^2: d=1 halves on DVE at 4x, d>=2 halves on ACT.
        IND = pool.tile([P, C, KW, H], BF16)
        IE = pool.tile([P, C, KW, H], BF16)
        ETP = pool.tile([P, KW, H], BF16)      # e_t (raw)
        ET2 = pool.tile([P, KW, H], BF16)
        TMPA = {}
        for d in range(1, K + 1):
            tmpa_d = pool.tile([P, C, KW, H], BF16, name=f"tmpa{d}")
            TMPA[d] = tmpa_d
        CUR = pool.tile([P, C, KW, H], BF16)
        R = pool.tile([P, C, KW, H], BF16)

        for half in range(2):
            cs = slice(2 * half, 2 * half + 2)
            # indicators for the e_t chain (Pool picks these up right away)
            nc.vector.tensor_scalar(IND[:, cs], D1T[:, cs], 0.0, None,
                                    op0=OP.is_equal)
            for c in range(2 * half, 2 * half + 2):
                nc.gpsimd.tensor_tensor(IE[:, c], IND[:, c], ET4[:, c],
                                        op=OP.mult)
            nc.vector.tensor_scalar(TMPA[1][:, cs], D1T[:, cs], 1.0, None,
                                    op0=OP.add)
            for d in range(2, K + 1):
                nc.scalar.activation(TMPA[d][:, cs], D1T[:, cs], ACT.Identity,
                                     bias=BIASQ[:, d - 1:d], scale=1.0)
            nc.vector.tensor_scalar(CUR[:, cs, :, H - 1:H],
                                    D1T[:, cs, :, H - 1:H], 0.0, None,
                                    op0=OP.add)
            nc.vector.tensor_tensor(CUR[:, cs, :, 0:H - 1],
                                    D1T[:, cs, :, 0:H - 1],
                                    TMPA[1][:, cs, :, 1:H], op=OP.min)
            nc.vector.tensor_tensor(CUR[:, cs, :, 1:H], CUR[:, cs, :, 1:H],
                                    TMPA[1][:, cs, :, 0:H - 1], op=OP.min)
            for d in range(2, K + 1):
                nc.vector.tensor_tensor(
                    CUR[:, cs, :, 0:H - d], CUR[:, cs, :, 0:H - d],
                    TMPA[d][:, cs, :, d:H], op=OP.min)
                nc.vector.tensor_tensor(
                    CUR[:, cs, :, d:H], CUR[:, cs, :, d:H],
                    TMPA[d][:, cs, :, 0:H - d], op=OP.min)
            nc.scalar.activation(R[:, cs], CUR[:, cs], ACT.Sqrt)

        # e_t chain tail on Pool (IE parts land while the col chains run)
        nc.gpsimd.tensor_tensor(ETP[:], IE[:, 0], IE[:, 1], op=OP.add)
        nc.gpsimd.tensor_tensor(ET2[:], IE[:, 2], IE[:, 3], op=OP.add)
        nc.gpsimd.tensor_tensor(ETP[:], ETP[:], ET2[:], op=OP.add)

        # ---- products for S = sum_c e_c R_c (G2/G3 on Pool) ----
        G0 = pool.tile([P, KW, H], BF16)
        G1 = pool.tile([P, KW, H], BF16)
        G2 = pool.tile([P, KW, H], BF16)
        G3 = pool.tile([P, KW, H], BF16)
        XM = pool.tile([P, KW, H], BF16)
        nc.gpsimd.tensor_tensor(G2[:], ET4[:, 2], R[:, 2], op=OP.mult)
        nc.gpsimd.tensor_tensor(G3[:], ET4[:, 3], R[:, 3], op=OP.mult)
        nc.gpsimd.tensor_tensor(G2[:], G2[:], G3[:], op=OP.add)
        nc.vector.tensor_tensor(G0[:], ET4[:, 0], R[:, 0], op=OP.mult)
        nc.vector.tensor_tensor(G1[:], ET4[:, 1], R[:, 1], op=OP.mult)
        nc.vector.tensor_tensor(G0[:], G0[:], G1[:], op=OP.add)

        # ---- order stats on R: m2 = second-smallest, cert = max ----
        A2 = pool.tile([P, KW, H], BF16)
        B2 = pool.tile([P, KW, H], BF16)
        C2 = pool.tile([P, KW, H], BF16)
        D2 = pool.tile([P, KW, H], BF16)
        M2 = pool.tile([P, KW, H], BF16)
        OUT = pool.tile([P, 4], FP32)
        nc.vector.tensor_tensor(A2[:], R[:, 0], R[:, 1], op=OP.min)
        nc.vector.tensor_tensor(B2[:], R[:, 0], R[:, 1], op=OP.max)
        nc.vector.tensor_tensor(C2[:], R[:, 2], R[:, 3], op=OP.min)
        nc.vector.tensor_tensor(D2[:], R[:, 2], R[:, 3], op=OP.max)
        nc.vector.tensor_tensor(M2[:], A2[:], C2[:], op=OP.max)
        nc.vector.tensor_tensor(C2[:], B2[:], D2[:], op=OP.min)
        nc.vector.tensor_tensor(B2[:], B2[:], D2[:], op=OP.max)  # max R
        nc.vector.tensor_tensor(M2[:], M2[:], C2[:], op=OP.min)  # secondmin
        nc.vector.tensor_tensor(XM[:], ETP[:], M2[:], op=OP.mult)
        nc.vector.tensor_tensor(G0[:], G0[:], G2[:], op=OP.add)
        JUNK = G1
        nc.vector.scalar_tensor_tensor(JUNK[:], G0[:], 1.0, RC[:],
                                       op0=OP.mult, op1=OP.mult,
                                       accum_out=OUT[:, 0:1])
        nc.vector.scalar_tensor_tensor(JUNK[:], XM[:], 1.0, RC[:],
                                       op0=OP.mult, op1=OP.mult,
                                       accum_out=OUT[:, 2:3])
        nc.vector.tensor_reduce(OUT[:, 1:2], B2[:], axis=mybir.AxisListType.XY,
                                op=OP.max)
        nc.vector.memset(OUT[:, 3:4], 0.0)
        nc.sync.dma_start(out_d[:], OUT[:])

    nc.finalize()
    return nc


_NC_CACHE: dict[int, bass.Bass] = {}


def _get_nc(K: int) -> bass.Bass:
    if K not in _NC_CACHE:
        _NC_CACHE[K] = _build_nc(K)
    return _NC_CACHE[K]


def _run_device(logits: np.ndarray, targets: np.ndarray, K: int, **kw):
    nc = _get_nc(K)
    in_maps = [
        {"logits": np.ascontiguousarray(logits[b], dtype=np.float32),
         "targets": np.ascontiguousarray(targets[b], dtype=np.int32)}
        for b in range(N_CORES)
    ]
    return run_bass_kernel_spmd(nc, in_maps, list(range(N_CORES)), **kw)


# ---------------------------------------------------------------------------
# exact host fallback (degenerate masks / failed certificate; ~never taken)
# ---------------------------------------------------------------------------

def _edt2_exact_np(mask: np.ndarray) -> np.ndarray:
    Hh, Ww = mask.shape
    f = np.where(mask, 0.0, 1e8)
    iw = np.arange(Ww, dtype=np.float64)
    sqw = (iw[:, None] - iw[None, :]) ** 2
    d1 = (f[:, None, :] + sqw[None, :, :]).min(axis=-1)
    ih = np.arange(Hh, dtype=np.float64)
    sqh = (ih[:, None] - ih[None, :]) ** 2
    d2 = (d1[None, :, :] + sqh[:, :, None]).min(axis=1)
    return d2


def _loss_host_exact(logits: np.ndarray, targets: np.ndarray) -> np.float32:
    B = logits.shape[0]
    lo = logits.astype(np.float64)
    mx = lo.max(axis=1, keepdims=True)
    e = np.exp(lo - mx)
    probs = e / e.sum(axis=1, keepdims=True)
    total = 0.0
    for b in range(B):
        for c in range(C):
            m = targets[b] == c
            s = int(m.sum())
            pos = np.sqrt(_edt2_exact_np(m))
            if s == 0:
                phi = pos
            elif s == m.size:
                phi = -np.sqrt(_edt2_exact_np(~m))
            else:
                phi = pos - np.sqrt(_edt2_exact_np(~m)) + 1.0
            total += float((probs[b, c] * phi).sum())
    return np.float32(total / (B * C * H * W))


def kernel(logits: np.ndarray, targets: np.ndarray) -> np.ndarray:
    logits = np.asarray(logits)
    targets = np.asarray(targets)
    assert logits.shape == (N_CORES, C, H, W) and targets.shape == (N_CORES, H, W)

    counts = np.stack([(targets == c).sum(axis=(1, 2)) for c in range(C)])
    if counts.min() == 0 or counts.max() == H * W:
        return np.asarray(_loss_host_exact(logits, targets))

    K = DEFAULT_K
    for _attempt in range(3):
        res = _run_device(logits, targets, K).results
        out = np.stack([res[b]["out"] for b in range(N_CORES)])  # (8, 128, 4)
        maxr = float(out[:, :, 1].max())
        # cert: every computed D with sqrt <= K+1 is provably exact
        if maxr <= (K + 1) + 1e-3:
            total = (float(out[:, :, 0].astype(np.float64).sum())
                     - float(out[:, :, 2].astype(np.float64).sum()))
            return np.asarray(
                np.float32(total / (N_CORES * C * H * W) + 1.0 / C))
        if maxr > 4000.0:  # sentinel leaked: a window saw no features
            break
        K = int(np.ceil(maxr))
    return np.asarray(_loss_host_exact(logits, targets))


# revision 12
# speedup vs baseline: 1.0643x; 1.0643x over previous
"""Boundary-loss Trainium2 kernel (Bass/Tile), SPMD over 8 NeuronCores.

loss = mean(softmax(logits, C) * phi(targets)), phi the signed EDT map.
Per pixel p with target t:  sum_c probs_c*phi_c = (sum_c e_c R_c - e_t*m2)/S_e + 1
with R_c = sqrt(edt2(mask_c)), m2 = min_{c!=t} R_c (= second-smallest R).

Device algorithm (one batch item per core, bf16 maps unless noted):
  * masks F_c = (t != c)*BIG with BIG pad columns; the 1-D L1 row distance
    runs as TWO flattened tensor_tensor_scans (fw + reversed bw) on DVE,
    then one strided row-min.
  * PE transposes logits (fp32) and rho (bf16) blockwise; ACT evicts PSUM
    fused with Exp (E = e^logit) resp. Square (d1 = rho^2). Everything
    downstream stays in transposed space -- no back-transposes.
  * col pass: exact windowed parabolic mins, window K: ACT/DVE prebuild
    TMPA_d = d1 + d^2, DVE runs one merged in-place 2x-mode min chain over
    all 4 classes (2 shifted mins per delta).
  * R = sqrt(D) on ACT; order stats run on R, so m2 needs no extra sqrt and
    the exactness certificate is max(R) <= K+1 (any pixel whose computed D
    is <= (K+1)^2 is provably exact; host retries with K+1 else).
  * tail on raw e_c (softmax never materialized): e_t via [d1==0]
    indicators (DVE 4x tensor_scalar) with mult/add chains on GPSIMD;
    S = sum_c e_c R_c; two fused scalar_tensor_tensor accumulations fold
    the single 1/S_e map into per-partition sums of S/S_e and e_t*m2/S_e;
    the host subtracts them and adds the +1/C term.
"""
from contextlib import ExitStack

import numpy as np

import concourse.bass as bass
import concourse.tile as tile
from concourse import bacc, mybir
from concourse.bass_utils import run_bass_kernel_spmd
from concourse.masks import make_identity

P = 128
C = 4
H = W = 384
KCH = H // P     # 3 row chunks (natural space)
KW = W // P      # 3 col chunks (transposed space)
PAD = 8
WP = W + PAD     # padded row length for the flattened scans
FLAT = C * KCH * WP
N_CORES = 8
BIG = 65536.0
DEFAULT_K = 4    # parabolic window; exact iff max R <= K+1 (certified)

FP32 = mybir.dt.float32
BF16 = mybir.dt.bfloat16
INT32 = mybir.dt.int32
OP = mybir.AluOpType
ACT = mybir.ActivationFunctionType


def _build_nc(K: int) -> bass.Bass:
    nc = bacc.Bacc("TRN2", target_bir_lowering=False, debug=False)
    logits_d = nc.dram_tensor("logits", [C, H, W], FP32, kind="ExternalInput")
    targets_d = nc.dram_tensor("targets", [H, W], INT32, kind="ExternalInput")
    out_d = nc.dram_tensor("out", [P, 4], FP32, kind="ExternalOutput")

    with tile.TileContext(nc) as tc, ExitStack() as ctx:
        pool = ctx.enter_context(tc.tile_pool(name="main", bufs=1))
        psq = ctx.enter_context(tc.tile_pool(name="psq", bufs=2, space="PSUM"))
        psl = ctx.enter_context(tc.tile_pool(name="psl", bufs=2, space="PSUM"))

        # ---- loads ----
        T = pool.tile([P, KCH, W], INT32)
        tr = targets_d[:].rearrange("(k p) w -> p k w", p=P)
        for k in range(KCH):
            nc.sync.dma_start(T[:, k], tr[:, k])
        L = pool.tile([P, C, KCH, W], FP32)
        nc.sync.dma_start(L[:], logits_d[:].rearrange("c (k p) w -> p c k w", p=P))

        # ---- constants ----
        ONES = pool.tile([P, 1], BF16)
        nc.vector.memset(ONES[:], 1.0)
        IDENT = pool.tile([P, P], BF16)
        make_identity(nc, IDENT[:])
        IDF = pool.tile([P, P], FP32)
        make_identity(nc, IDF[:])
        BIASQ = pool.tile([P, K], FP32)
        nc.vector.memset(BIASQ[:, 0:1], 1.0)
        for d in range(2, K + 1):
            nc.vector.memset(BIASQ[:, d - 1:d], float(d * d))

        # ---- masks F_c = (t != c)*BIG, with BIG pad columns ----
        F = pool.tile([P, C, KCH, WP], BF16)
        nc.gpsimd.memset(F[:, :, :, W:WP], BIG)
        TFb = pool.tile([P, KCH, W], BF16)
        for k in range(KCH):
            nc.scalar.copy(TFb[:, k], T[:, k])
        for c in range(C):
            nc.vector.tensor_scalar(F[:, c, :, 0:W], TFb[:], float(c), BIG,
                                    op0=OP.not_equal, op1=OP.mult)

        # ---- row pass: flattened L1 scans, then min ----
        FW = pool.tile([P, C, KCH, WP], BF16)
        BW = pool.tile([P, C, KCH, WP], BF16)
        ff = F[:].rearrange("p c k w -> p (c k w)")
        fwf = FW[:].rearrange("p c k w -> p (c k w)")
        bwf = BW[:].rearrange("p c k w -> p (c k w)")
        HFLAT = 2 * KCH * WP
        ONESB = ONES[:, 0:1].broadcast_to([P, HFLAT])
        for h in range(2):
            lo, hi = h * HFLAT, (h + 1) * HFLAT
            nc.vector.tensor_tensor_scan(fwf[:, lo:hi], ONESB, ff[:, lo:hi],
                                         BIG, op0=OP.add, op1=OP.min)
            nc.vector.tensor_tensor_scan(bwf[:, lo:hi][:, ::-1], ONESB,
                                         ff[:, lo:hi][:, ::-1], BIG,
                                         op0=OP.add, op1=OP.min)
            nc.vector.tensor_tensor(FW[:, 2 * h:2 * h + 2, :, 0:W],
                                    FW[:, 2 * h:2 * h + 2, :, 0:W],
                                    BW[:, 2 * h:2 * h + 2, :, 0:W],
                                    op=OP.min)  # rho

        # ---- PE transposes + fused evictions ----
        # logits first (PE is idle while scans run; E feeds the mid chains),
        # then rho per class (feeds the col pass).
        ET4 = pool.tile([P, C, KW, H], BF16)   # e^logit, transposed
        D1T = pool.tile([P, C, KW, H], BF16)   # rho^2, transposed
        for c in range(C):
            for kw in range(KW):
                pl = psl.tile([P, KCH, P], FP32, tag="psl")
                for kh in range(KCH):
                    nc.tensor.matmul(pl[:, kh, :],
                                     L[:, c, kh, kw * P:(kw + 1) * P],
                                     IDF[:], is_transpose=True)
                nc.scalar.activation(
                    ET4[:, c, kw], pl[:].rearrange("p kh x -> p (kh x)"),
                    ACT.Exp)
        for c in range(C):
            p9 = psq.tile([P, KW, KCH, P], BF16, tag="ps9")
            for kw in range(KW):
                for kh in range(KCH):
                    nc.tensor.matmul(p9[:, kw, kh, :],
                                     FW[:, c, kh, kw * P:(kw + 1) * P],
                                     IDENT[:], is_transpose=True)
            nc.scalar.activation(
                D1T[:, c], p9[:].rearrange("p kw kh x -> p kw (kh x)"),
                ACT.Square)

        # ---- S_e and 1/S_e (fills the DVE gap while rho transposes run;
        # 1/S_e folds into the two final fused accumulations) ----
        SE = pool.tile([P, KW, H], BF16)
        TMP = pool.tile([P, KW, H], BF16)
        nc.gpsimd.tensor_tensor(SE[:], ET4[:, 0], ET4[:, 1], op=OP.add)
        nc.gpsimd.tensor_tensor(TMP[:], ET4[:, 2], ET4[:, 3], op=OP.add)
        nc.gpsimd.tensor_tensor(SE[:], SE[:], TMP[:], op=OP.add)
        RC = pool.tile([P, KW, H], FP32)
        nc.vector.reciprocal(RC[:], SE[:])

        # ---- col pass, split into two independent 2-class chains so the
        # first half's sqrt/stats/products start while the second half runs.
        # TMPA_d = d1 + d^2: d=1 halves on DVE at 4x, d>=2 halves on ACT.
        IND = pool.tile([P, C, KW, H], BF16)
        IE = pool.tile([P, C, KW, H], BF16)
        ETP = pool.tile([P, KW, H], BF16)      # e_t (raw)
        ET2 = pool.tile([P, KW, H], BF16)
        TMPA = {}
        for d in range(1, K + 1):
            tmpa_d = pool.tile([P, C, KW, H], BF16, name=f"tmpa{d}")
            TMPA[d] = tmpa_d
        CUR = pool.tile([P, C, KW, H], BF16)
        R = pool.tile([P, C, KW, H], BF16)

        for half in range(2):
            cs = slice(2 * half, 2 * half + 2)
            # indicators for the e_t chain (Pool picks these up right away)
            nc.vector.tensor_scalar(IND[:, cs], D1T[:, cs], 0.0, None,
                                    op0=OP.is_equal)
            for c in range(2 * half, 2 * half + 2):
                nc.gpsimd.tensor_tensor(IE[:, c], IND[:, c], ET4[:, c],
                                        op=OP.mult)
            nc.scalar.activation(TMPA[1][:, cs], D1T[:, cs], ACT.Identity,
                                 bias=BIASQ[:, 0:1], scale=1.0)
            for d in range(2, K + 1):
                nc.scalar.activation(TMPA[d][:, cs], D1T[:, cs], ACT.Identity,
                                     bias=BIASQ[:, d - 1:d], scale=1.0)
            nc.vector.tensor_scalar(CUR[:, cs, :, H - 1:H],
                                    D1T[:, cs, :, H - 1:H], 0.0, None,
                                    op0=OP.add)
            nc.vector.tensor_tensor(CUR[:, cs, :, 0:H - 1],
                                    D1T[:, cs, :, 0:H - 1],
                                    TMPA[1][:, cs, :, 1:H], op=OP.min)
            nc.vector.tensor_tensor(CUR[:, cs, :, 1:H], CUR[:, cs, :, 1:H],
                                    TMPA[1][:, cs, :, 0:H - 1], op=OP.min)
            for d in range(2, K + 1):
                nc.vector.tensor_tensor(
                    CUR[:, cs, :, 0:H - d], CUR[:, cs, :, 0:H - d],
                    TMPA[d][:, cs, :, d:H], op=OP.min)
                nc.vector.tensor_tensor(
                    CUR[:, cs, :, d:H], CUR[:, cs, :, d:H],
                    TMPA[d][:, cs, :, 0:H - d], op=OP.min)
            nc.scalar.activation(R[:, cs], CUR[:, cs], ACT.Sqrt)

        # e_t chain tail on Pool (IE parts land while the col chains run)
        nc.gpsimd.tensor_tensor(ETP[:], IE[:, 0], IE[:, 1], op=OP.add)
        nc.gpsimd.tensor_tensor(ET2[:], IE[:, 2], IE[:, 3], op=OP.add)
        nc.gpsimd.tensor_tensor(ETP[:], ETP[:], ET2[:], op=OP.add)

        # ---- products for S = sum_c e_c R_c (second half on Pool) ----
        G0 = pool.tile([P, KW, H], BF16)
        G1 = pool.tile([P, KW, H], BF16)
        G2 = pool.tile([P, KW, H], BF16)
        G3 = pool.tile([P, KW, H], BF16)
        XM = pool.tile([P, KW, H], BF16)
        nc.vector.tensor_tensor(G0[:], ET4[:, 0], R[:, 0], op=OP.mult)
        nc.vector.tensor_tensor(G1[:], ET4[:, 1], R[:, 1], op=OP.mult)
        nc.vector.tensor_tensor(G0[:], G0[:], G1[:], op=OP.add)
        nc.gpsimd.tensor_tensor(G2[:], ET4[:, 2], R[:, 2], op=OP.mult)
        nc.gpsimd.tensor_tensor(G3[:], ET4[:, 3], R[:, 3], op=OP.mult)
        nc.gpsimd.tensor_tensor(G2[:], G2[:], G3[:], op=OP.add)

        # ---- order stats on D (no sqrt wait): m2^2, cert = max D ----
        A2 = pool.tile([P, KW, H], BF16)
        B2 = pool.tile([P, KW, H], BF16)
        C2 = pool.tile([P, KW, H], BF16)
        D2 = pool.tile([P, KW, H], BF16)
        M2D = pool.tile([P, KW, H], BF16)
        M2 = pool.tile([P, KW, H], BF16)
        OUT = pool.tile([P, 4], FP32)
        nc.vector.tensor_tensor(A2[:], CUR[:, 0], CUR[:, 1], op=OP.min)
        nc.vector.tensor_tensor(B2[:], CUR[:, 0], CUR[:, 1], op=OP.max)
        nc.vector.tensor_tensor(C2[:], CUR[:, 2], CUR[:, 3], op=OP.min)
        nc.vector.tensor_tensor(D2[:], CUR[:, 2], CUR[:, 3], op=OP.max)
        nc.vector.tensor_tensor(M2D[:], A2[:], C2[:], op=OP.max)
        nc.vector.tensor_tensor(C2[:], B2[:], D2[:], op=OP.min)
        nc.vector.tensor_tensor(B2[:], B2[:], D2[:], op=OP.max)  # max D
        nc.vector.tensor_tensor(M2D[:], M2D[:], C2[:], op=OP.min)  # 2nd-min
        nc.vector.tensor_reduce(OUT[:, 1:2], B2[:], axis=mybir.AxisListType.XY,
                                op=OP.max)
        nc.scalar.activation(M2[:], M2D[:], ACT.Sqrt)

        # ---- accumulate S/S_e and e_t*m2/S_e ----
        nc.vector.tensor_tensor(XM[:], ETP[:], M2[:], op=OP.mult)
        nc.vector.tensor_tensor(G0[:], G0[:], G2[:], op=OP.add)
        JUNK = G1
        nc.vector.scalar_tensor_tensor(JUNK[:], G0[:], 1.0, RC[:],
                                       op0=OP.mult, op1=OP.mult,
                                       accum_out=OUT[:, 0:1])
        nc.vector.scalar_tensor_tensor(JUNK[:], XM[:], 1.0, RC[:],
                                       op0=OP.mult, op1=OP.mult,
                                       accum_out=OUT[:, 2:3])
        nc.vector.memset(OUT[:, 3:4], 0.0)
        nc.sync.dma_start(out_d[:], OUT[:])

    nc.finalize()
    return nc


_NC_CACHE: dict[int, bass.Bass] = {}


def _get_nc(K: int) -> bass.Bass:
    if K not in _NC_CACHE:
        _NC_CACHE[K] = _build_nc(K)
    return _NC_CACHE[K]


def _run_device(logits: np.ndarray, targets: np.ndarray, K: int, **kw):
    nc = _get_nc(K)
    in_maps = [
        {"logits": np.ascontiguousarray(logits[b], dtype=np.float32),
         "targets": np.ascontiguousarray(targets[b], dtype=np.int32)}
        for b in range(N_CORES)
    ]
    return run_bass_kernel_spmd(nc, in_maps, list(range(N_CORES)), **kw)


# ---------------------------------------------------------------------------
# exact host fallback (degenerate masks / failed certificate; ~never taken)
# ---------------------------------------------------------------------------

def _edt2_exact_np(mask: np.ndarray) -> np.ndarray:
    Hh, Ww = mask.shape
    f = np.where(mask, 0.0, 1e8)
    iw = np.arange(Ww, dtype=np.float64)
    sqw = (iw[:, None] - iw[None, :]) ** 2
    d1 = (f[:, None, :] + sqw[None, :, :]).min(axis=-1)
    ih = np.arange(Hh, dtype=np.float64)
    sqh = (ih[:, None] - ih[None, :]) ** 2
    d2 = (d1[None, :, :] + sqh[:, :, None]).min(axis=1)
    return d2


def _loss_host_exact(logits: np.ndarray, targets: np.ndarray) -> np.float32:
    B = logits.shape[0]
    lo = logits.astype(np.float64)
    mx = lo.max(axis=1, keepdims=True)
    e = np.exp(lo - mx)
    probs = e / e.sum(axis=1, keepdims=True)
    total = 0.0
    for b in range(B):
        for c in range(C):
            m = targets[b] == c
            s = int(m.sum())
            pos = np.sqrt(_edt2_exact_np(m))
            if s == 0:
                phi = pos
            elif s == m.size:
                phi = -np.sqrt(_edt2_exact_np(~m))
            else:
                phi = pos - np.sqrt(_edt2_exact_np(~m)) + 1.0
            total += float((probs[b, c] * phi).sum())
    return np.float32(total / (B * C * H * W))


def kernel(logits: np.ndarray, targets: np.ndarray) -> np.ndarray:
    logits = np.asarray(logits)
    targets = np.asarray(targets)
    assert logits.shape == (N_CORES, C, H, W) and targets.shape == (N_CORES, H, W)

    counts = np.stack([(targets == c).sum(axis=(1, 2)) for c in range(C)])
    if counts.min() == 0 or counts.max() == H * W:
        return np.asarray(_loss_host_exact(logits, targets))

    K = DEFAULT_K
    for _attempt in range(3):
        res = _run_device(logits, targets, K).results
        out = np.stack([res[b]["out"] for b in range(N_CORES)])  # (8, 128, 4)
        maxd = float(out[:, :, 1].max())
        # cert: every computed D <= (K+1)^2 is provably exact
        if maxd <= (K + 1) ** 2 + 0.5:
            total = (float(out[:, :, 0].astype(np.float64).sum())
                     - float(out[:, :, 2].astype(np.float64).sum()))
            return np.asarray(
                np.float32(total / (N_CORES * C * H * W) + 1.0 / C))
        if maxd > 4000.0 ** 2:  # sentinel leaked: a window saw no features
            break
        K = int(np.ceil(np.sqrt(maxd)))
    return np.asarray(_loss_host_exact(logits, targets))


# revision 13
# speedup vs baseline: 1.1372x; 1.0685x over previous
"""Boundary-loss Trainium2 kernel (Bass/Tile), SPMD over 8 NeuronCores.

loss = mean(softmax(logits, C) * phi(targets)), phi the signed EDT map.
Per pixel p with target t:  sum_c probs_c*phi_c = (sum_c e_c R_c - e_t*m2)/S_e + 1
with R_c = sqrt(edt2(mask_c)), m2 = min_{c!=t} R_c (= second-smallest R).

Device algorithm (one batch item per core, bf16 maps unless noted):
  * masks F_c = (t != c)*BIG with BIG pad columns; the 1-D L1 row distance
    runs as TWO flattened tensor_tensor_scans (fw + reversed bw) on DVE,
    then one strided row-min.
  * PE transposes logits (fp32) and rho (bf16) blockwise; ACT evicts PSUM
    fused with Exp (E = e^logit) resp. Square (d1 = rho^2). Everything
    downstream stays in transposed space -- no back-transposes.
  * col pass: exact windowed parabolic mins, window K: ACT/DVE prebuild
    TMPA_d = d1 + d^2, DVE runs one merged in-place 2x-mode min chain over
    all 4 classes (2 shifted mins per delta).
  * R = sqrt(D) on ACT; order stats run on R, so m2 needs no extra sqrt and
    the exactness certificate is max(R) <= K+1 (any pixel whose computed D
    is <= (K+1)^2 is provably exact; host retries with K+1 else).
  * tail on raw e_c (softmax never materialized): e_t via [d1==0]
    indicators (DVE 4x tensor_scalar) with mult/add chains on GPSIMD;
    S = sum_c e_c R_c; two fused scalar_tensor_tensor accumulations fold
    the single 1/S_e map into per-partition sums of S/S_e and e_t*m2/S_e;
    the host subtracts them and adds the +1/C term.
"""
from contextlib import ExitStack

import numpy as np

import concourse.bass as bass
import concourse.tile as tile
from concourse import bacc, mybir
from concourse.bass_utils import run_bass_kernel_spmd
from concourse.masks import make_identity

P = 128
C = 4
H = W = 384
KCH = H // P     # 3 row chunks (natural space)
KW = W // P      # 3 col chunks (transposed space)
PAD = 8
WP = W + PAD     # padded row length for the flattened scans
FLAT = C * KCH * WP
N_CORES = 8
BIG = 65536.0
DEFAULT_K = 4    # parabolic window; exact iff max R <= K+1 (certified)

FP32 = mybir.dt.float32
BF16 = mybir.dt.bfloat16
INT32 = mybir.dt.int32
OP = mybir.AluOpType
ACT = mybir.ActivationFunctionType


def _build_nc(K: int) -> bass.Bass:
    nc = bacc.Bacc("TRN2", target_bir_lowering=False, debug=False)
    logits_d = nc.dram_tensor("logits", [C, H, W], FP32, kind="ExternalInput")
    targets_d = nc.dram_tensor("targets", [H, W], INT32, kind="ExternalInput")
    out_d = nc.dram_tensor("out", [P, 4], FP32, kind="ExternalOutput")

    with tile.TileContext(nc) as tc, ExitStack() as ctx:
        pool = ctx.enter_context(tc.tile_pool(name="main", bufs=1))
        psq = ctx.enter_context(tc.tile_pool(name="psq", bufs=2, space="PSUM"))
        psl = ctx.enter_context(tc.tile_pool(name="psl", bufs=2, space="PSUM"))

        # ---- loads ----
        T = pool.tile([P, KCH, W], INT32)
        tr = targets_d[:].rearrange("(k p) w -> p k w", p=P)
        for k in range(KCH):
            nc.sync.dma_start(T[:, k], tr[:, k])
        L = pool.tile([P, C, KCH, W], FP32)
        lview = logits_d[:].rearrange("c (k p) w -> p c k w", p=P)
        nc.sync.dma_start(L[:, 0:2], lview[:, 0:2])
        nc.sync.dma_start(L[:, 2:4], lview[:, 2:4])

        # ---- constants ----
        ONES = pool.tile([P, 1], BF16)
        nc.vector.memset(ONES[:], 1.0)
        IDENT = pool.tile([P, P], BF16)
        make_identity(nc, IDENT[:])
        IDF = pool.tile([P, P], FP32)
        make_identity(nc, IDF[:])
        BIASQ = pool.tile([P, K], FP32)
        nc.vector.memset(BIASQ[:, 0:1], 1.0)
        for d in range(2, K + 1):
            nc.vector.memset(BIASQ[:, d - 1:d], float(d * d))

        # ---- masks F_c = (t != c)*BIG, with BIG pad columns ----
        F = pool.tile([P, C, KCH, WP], BF16)
        nc.gpsimd.memset(F[:, :, :, W:WP], BIG)
        TFb = pool.tile([P, KCH, W], BF16)
        for k in range(KCH):
            nc.scalar.copy(TFb[:, k], T[:, k])
        for c in range(C):
            nc.vector.tensor_scalar(F[:, c, :, 0:W], TFb[:], float(c), BIG,
                                    op0=OP.not_equal, op1=OP.mult)

        # ---- row pass: flattened L1 scans, then min ----
        FW = pool.tile([P, C, KCH, WP], BF16)
        BW = pool.tile([P, C, KCH, WP], BF16)
        ff = F[:].rearrange("p c k w -> p (c k w)")
        fwf = FW[:].rearrange("p c k w -> p (c k w)")
        bwf = BW[:].rearrange("p c k w -> p (c k w)")
        HFLAT = 2 * KCH * WP
        ONESB = ONES[:, 0:1].broadcast_to([P, HFLAT])
        for h in range(2):
            lo, hi = h * HFLAT, (h + 1) * HFLAT
            nc.vector.tensor_tensor_scan(fwf[:, lo:hi], ONESB, ff[:, lo:hi],
                                         BIG, op0=OP.add, op1=OP.min)
            nc.vector.tensor_tensor_scan(bwf[:, lo:hi][:, ::-1], ONESB,
                                         ff[:, lo:hi][:, ::-1], BIG,
                                         op0=OP.add, op1=OP.min)
            nc.vector.tensor_tensor(FW[:, 2 * h:2 * h + 2, :, 0:W],
                                    FW[:, 2 * h:2 * h + 2, :, 0:W],
                                    BW[:, 2 * h:2 * h + 2, :, 0:W],
                                    op=OP.min)  # rho

        # ---- PE transposes + fused evictions ----
        # logits first (PE is idle while scans run; E feeds the mid chains),
        # then rho per class (feeds the col pass).
        ET4 = pool.tile([P, C, KW, H], BF16)   # e^logit, transposed
        D1T = pool.tile([P, C, KW, H], BF16)   # rho^2, transposed
        for c in range(C):
            for kw in range(KW):
                pl = psl.tile([P, KCH, P], FP32, tag="psl")
                for kh in range(KCH):
                    nc.tensor.matmul(pl[:, kh, :],
                                     L[:, c, kh, kw * P:(kw + 1) * P],
                                     IDF[:], is_transpose=True)
                nc.scalar.activation(
                    ET4[:, c, kw], pl[:].rearrange("p kh x -> p (kh x)"),
                    ACT.Exp)
        for c in range(C):
            p9 = psq.tile([P, KW, KCH, P], BF16, tag="ps9")
            for kw in range(KW):
                for kh in range(KCH):
                    nc.tensor.matmul(p9[:, kw, kh, :],
                                     FW[:, c, kh, kw * P:(kw + 1) * P],
                                     IDENT[:], is_transpose=True)
            nc.scalar.activation(
                D1T[:, c], p9[:].rearrange("p kw kh x -> p kw (kh x)"),
                ACT.Square)

        # ---- S_e and 1/S_e (fills the DVE gap while rho transposes run;
        # 1/S_e folds into the two final fused accumulations) ----
        SE = pool.tile([P, KW, H], BF16)
        TMP = pool.tile([P, KW, H], BF16)
        nc.gpsimd.tensor_tensor(SE[:], ET4[:, 0], ET4[:, 1], op=OP.add)
        nc.gpsimd.tensor_tensor(TMP[:], ET4[:, 2], ET4[:, 3], op=OP.add)
        nc.gpsimd.tensor_tensor(SE[:], SE[:], TMP[:], op=OP.add)
        RC = pool.tile([P, KW, H], FP32)
        nc.vector.reciprocal(RC[:], SE[:])

        # ---- col pass, split into two independent 2-class chains so the
        # first half's sqrt/stats/products start while the second half runs.
        # TMPA_d = d1 + d^2: d=1 halves on DVE at 4x, d>=2 halves on ACT.
        IND = pool.tile([P, C, KW, H], BF16)
        IE = pool.tile([P, C, KW, H], BF16)
        ETP = pool.tile([P, KW, H], BF16)      # e_t (raw)
        ET2 = pool.tile([P, KW, H], BF16)
        TMPA = {}
        for d in range(1, K + 1):
            tmpa_d = pool.tile([P, C, KW, H], BF16, name=f"tmpa{d}")
            TMPA[d] = tmpa_d
        CUR = pool.tile([P, C, KW, H], BF16)
        R = pool.tile([P, C, KW, H], BF16)

        for half in range(2):
            cs = slice(2 * half, 2 * half + 2)
            # indicators for the e_t chain (Pool picks these up right away)
            nc.vector.tensor_scalar(IND[:, cs], D1T[:, cs], 0.0, None,
                                    op0=OP.is_equal)
            for c in range(2 * half, 2 * half + 2):
                nc.gpsimd.tensor_tensor(IE[:, c], IND[:, c], ET4[:, c],
                                        op=OP.mult)
            nc.scalar.activation(TMPA[1][:, cs], D1T[:, cs], ACT.Identity,
                                 bias=BIASQ[:, 0:1], scale=1.0)
            for d in range(2, K + 1):
                nc.scalar.activation(TMPA[d][:, cs], D1T[:, cs], ACT.Identity,
                                     bias=BIASQ[:, d - 1:d], scale=1.0)
            nc.vector.tensor_scalar(CUR[:, cs, :, H - 1:H],
                                    D1T[:, cs, :, H - 1:H], 0.0, None,
                                    op0=OP.add)
            nc.vector.tensor_tensor(CUR[:, cs, :, 0:H - 1],
                                    D1T[:, cs, :, 0:H - 1],
                                    TMPA[1][:, cs, :, 1:H], op=OP.min)
            nc.vector.tensor_tensor(CUR[:, cs, :, 1:H], CUR[:, cs, :, 1:H],
                                    TMPA[1][:, cs, :, 0:H - 1], op=OP.min)
            for d in range(2, K + 1):
                nc.vector.tensor_tensor(
                    CUR[:, cs, :, 0:H - d], CUR[:, cs, :, 0:H - d],
                    TMPA[d][:, cs, :, d:H], op=OP.min)
                nc.vector.tensor_tensor(
                    CUR[:, cs, :, d:H], CUR[:, cs, :, d:H],
                    TMPA[d][:, cs, :, 0:H - d], op=OP.min)
            nc.scalar.activation(R[:, cs], CUR[:, cs], ACT.Sqrt)

        # e_t chain tail on Pool (IE parts land while the col chains run)
        nc.gpsimd.tensor_tensor(ETP[:], IE[:, 0], IE[:, 1], op=OP.add)
        nc.gpsimd.tensor_tensor(ET2[:], IE[:, 2], IE[:, 3], op=OP.add)
        nc.gpsimd.tensor_tensor(ETP[:], ETP[:], ET2[:], op=OP.add)

        # ---- products for S = sum_c e_c R_c (second half on Pool) ----
        G0 = pool.tile([P, KW, H], BF16)
        G1 = pool.tile([P, KW, H], BF16)
        G2 = pool.tile([P, KW, H], BF16)
        G3 = pool.tile([P, KW, H], BF16)
        XM = pool.tile([P, KW, H], BF16)
        nc.vector.tensor_tensor(G0[:], ET4[:, 0], R[:, 0], op=OP.mult)
        nc.vector.tensor_tensor(G1[:], ET4[:, 1], R[:, 1], op=OP.mult)
        nc.vector.tensor_tensor(G0[:], G0[:], G1[:], op=OP.add)
        nc.vector.tensor_tensor(G2[:], ET4[:, 2], R[:, 2], op=OP.mult)
        nc.vector.tensor_tensor(G3[:], ET4[:, 3], R[:, 3], op=OP.mult)
        nc.vector.tensor_tensor(G2[:], G2[:], G3[:], op=OP.add)

        # ---- order stats on D (no sqrt wait): m2^2, cert = max D ----
        A2 = pool.tile([P, KW, H], BF16)
        B2 = pool.tile([P, KW, H], BF16)
        C2 = pool.tile([P, KW, H], BF16)
        D2 = pool.tile([P, KW, H], BF16)
        M2D = pool.tile([P, KW, H], BF16)
        M2 = pool.tile([P, KW, H], BF16)
        OUT = pool.tile([P, 4], FP32)
        nc.vector.tensor_tensor(A2[:], CUR[:, 0], CUR[:, 1], op=OP.min)
        nc.vector.tensor_tensor(B2[:], CUR[:, 0], CUR[:, 1], op=OP.max)
        nc.vector.tensor_tensor(C2[:], CUR[:, 2], CUR[:, 3], op=OP.min)
        nc.vector.tensor_tensor(D2[:], CUR[:, 2], CUR[:, 3], op=OP.max)
        nc.vector.tensor_tensor(M2D[:], A2[:], C2[:], op=OP.max)
        nc.vector.tensor_tensor(C2[:], B2[:], D2[:], op=OP.min)
        nc.vector.tensor_tensor(B2[:], B2[:], D2[:], op=OP.max)  # max D
        nc.vector.tensor_tensor(M2D[:], M2D[:], C2[:], op=OP.min)  # 2nd-min
        nc.vector.tensor_reduce(OUT[:, 1:2], B2[:], axis=mybir.AxisListType.XY,
                                op=OP.max)
        nc.scalar.activation(M2[:], M2D[:], ACT.Sqrt)

        # ---- accumulate S/S_e and e_t*m2/S_e ----
        nc.vector.tensor_tensor(XM[:], ETP[:], M2[:], op=OP.mult)
        nc.vector.tensor_tensor(G0[:], G0[:], G2[:], op=OP.add)
        JUNK = G1
        nc.vector.scalar_tensor_tensor(JUNK[:], G0[:], 1.0, RC[:],
                                       op0=OP.mult, op1=OP.mult,
                                       accum_out=OUT[:, 0:1])
        nc.vector.scalar_tensor_tensor(JUNK[:], XM[:], 1.0, RC[:],
                                       op0=OP.mult, op1=OP.mult,
                                       accum_out=OUT[:, 2:3])
        nc.vector.memset(OUT[:, 3:4], 0.0)
        nc.sync.dma_start(out_d[:], OUT[:])

    nc.finalize()
    return nc


_NC_CACHE: dict[int, bass.Bass] = {}


def _get_nc(K: int) -> bass.Bass:
    if K not in _NC_CACHE:
        _NC_CACHE[K] = _build_nc(K)
    return _NC_CACHE[K]


def _run_device(logits: np.ndarray, targets: np.ndarray, K: int, **kw):
    nc = _get_nc(K)
    in_maps = [
        {"logits": np.ascontiguousarray(logits[b], dtype=np.float32),
         "targets": np.ascontiguousarray(targets[b], dtype=np.int32)}
        for b in range(N_CORES)
    ]
    return run_bass_kernel_spmd(nc, in_maps, list(range(N_CORES)), **kw)


# ---------------------------------------------------------------------------
# exact host fallback (degenerate masks / failed certificate; ~never taken)
# ---------------------------------------------------------------------------

def _edt2_exact_np(mask: np.ndarray) -> np.ndarray:
    Hh, Ww = mask.shape
    f = np.where(mask, 0.0, 1e8)
    iw = np.arange(Ww, dtype=np.float64)
    sqw = (iw[:, None] - iw[None, :]) ** 2
    d1 = (f[:, None, :] + sqw[None, :, :]).min(axis=-1)
    ih = np.arange(Hh, dtype=np.float64)
    sqh = (ih[:, None] - ih[None, :]) ** 2
    d2 = (d1[None, :, :] + sqh[:, :, None]).min(axis=1)
    return d2


def _loss_host_exact(logits: np.ndarray, targets: np.ndarray) -> np.float32:
    B = logits.shape[0]
    lo = logits.astype(np.float64)
    mx = lo.max(axis=1, keepdims=True)
    e = np.exp(lo - mx)
    probs = e / e.sum(axis=1, keepdims=True)
    total = 0.0
    for b in range(B):
        for c in range(C):
            m = targets[b] == c
            s = int(m.sum())
            pos = np.sqrt(_edt2_exact_np(m))
            if s == 0:
                phi = pos
            elif s == m.size:
                phi = -np.sqrt(_edt2_exact_np(~m))
            else:
                phi = pos - np.sqrt(_edt2_exact_np(~m)) + 1.0
            total += float((probs[b, c] * phi).sum())
    return np.float32(total / (B * C * H * W))


def kernel(logits: np.ndarray, targets: np.ndarray) -> np.ndarray:
    logits = np.asarray(logits)
    targets = np.asarray(targets)
    assert logits.shape == (N_CORES, C, H, W) and targets.shape == (N_CORES, H, W)

    counts = np.stack([(targets == c).sum(axis=(1, 2)) for c in range(C)])
    if counts.min() == 0 or counts.max() == H * W:
        return np.asarray(_loss_host_exact(logits, targets))

    K = DEFAULT_K
    for _attempt in range(3):
        res = _run_device(logits, targets, K).results
        out = np.stack([res[b]["out"] for b in range(N_CORES)])  # (8, 128, 4)
        maxd = float(out[:, :, 1].max())
        # cert: every computed D <= (K+1)^2 is provably exact
        if maxd <= (K + 1) ** 2 + 0.5:
            total = (float(out[:, :, 0].astype(np.float64).sum())
                     - float(out[:, :, 2].astype(np.float64).sum()))
            return np.asarray(
                np.float32(total / (N_CORES * C * H * W) + 1.0 / C))
        if maxd > 4000.0 ** 2:  # sentinel leaked: a window saw no features
            break
        K = int(np.ceil(np.sqrt(maxd)))
    return np.asarray(_loss_host_exact(logits, targets))


# revision 16
# speedup vs baseline: 1.1400x; 1.0024x over previous
"""Boundary-loss Trainium2 kernel (Bass/Tile), SPMD over 8 NeuronCores.

loss = mean(softmax(logits, C) * phi(targets)), phi the signed EDT map.
Per pixel p with target t:  sum_c probs_c*phi_c = (sum_c e_c R_c - e_t*m2)/S_e + 1
with R_c = sqrt(edt2(mask_c)), m2 = min_{c!=t} R_c (= second-smallest R).

Device algorithm (one batch item per core, bf16 maps unless noted):
  * masks F_c = (t != c)*BIG with BIG pad columns; the 1-D L1 row distance
    runs as TWO flattened tensor_tensor_scans (fw + reversed bw) on DVE,
    then one strided row-min.
  * PE transposes logits (fp32) and rho (bf16) blockwise; ACT evicts PSUM
    fused with Exp (E = e^logit) resp. Square (d1 = rho^2). Everything
    downstream stays in transposed space -- no back-transposes.
  * col pass: exact windowed parabolic mins, window K: ACT/DVE prebuild
    TMPA_d = d1 + d^2, DVE runs one merged in-place 2x-mode min chain over
    all 4 classes (2 shifted mins per delta).
  * R = sqrt(D) on ACT; order stats run on R, so m2 needs no extra sqrt and
    the exactness certificate is max(R) <= K+1 (any pixel whose computed D
    is <= (K+1)^2 is provably exact; host retries with K+1 else).
  * tail on raw e_c (softmax never materialized): e_t via [d1==0]
    indicators (DVE 4x tensor_scalar) with mult/add chains on GPSIMD;
    S = sum_c e_c R_c; two fused scalar_tensor_tensor accumulations fold
    the single 1/S_e map into per-partition sums of S/S_e and e_t*m2/S_e;
    the host subtracts them and adds the +1/C term.
"""
from contextlib import ExitStack

import numpy as np

import concourse.bass as bass
import concourse.tile as tile
from concourse import bacc, mybir
from concourse.bass_utils import run_bass_kernel_spmd
from concourse.masks import make_identity

P = 128
C = 4
H = W = 384
KCH = H // P     # 3 row chunks (natural space)
KW = W // P      # 3 col chunks (transposed space)
PAD = 8
WP = W + PAD     # padded row length for the flattened scans
FLAT = C * KCH * WP
N_CORES = 8
BIG = 65536.0
DEFAULT_K = 4    # parabolic window; exact iff max R <= K+1 (certified)

FP32 = mybir.dt.float32
BF16 = mybir.dt.bfloat16
INT32 = mybir.dt.int32
OP = mybir.AluOpType
ACT = mybir.ActivationFunctionType


def _build_nc(K: int) -> bass.Bass:
    nc = bacc.Bacc("TRN2", target_bir_lowering=False, debug=False)
    logits_d = nc.dram_tensor("logits", [C, H, W], FP32, kind="ExternalInput")
    targets_d = nc.dram_tensor("targets", [H, W], INT32, kind="ExternalInput")
    out_d = nc.dram_tensor("out", [P, 4], FP32, kind="ExternalOutput")

    with tile.TileContext(nc) as tc, ExitStack() as ctx:
        pool = ctx.enter_context(tc.tile_pool(name="main", bufs=1))
        psq = ctx.enter_context(tc.tile_pool(name="psq", bufs=2, space="PSUM"))
        psl = ctx.enter_context(tc.tile_pool(name="psl", bufs=2, space="PSUM"))

        # ---- loads ----
        T = pool.tile([P, KCH, W], INT32)
        tr = targets_d[:].rearrange("(k p) w -> p k w", p=P)
        for k in range(KCH):
            nc.sync.dma_start(T[:, k], tr[:, k])
        L = pool.tile([P, C, KCH, W], FP32)
        lview = logits_d[:].rearrange("c (k p) w -> p c k w", p=P)
        nc.sync.dma_start(L[:, 0:2], lview[:, 0:2])
        nc.sync.dma_start(L[:, 2:4], lview[:, 2:4])

        # ---- constants ----
        ONES = pool.tile([P, 1], BF16)
        nc.vector.memset(ONES[:], 1.0)
        IDENT = pool.tile([P, P], BF16)
        make_identity(nc, IDENT[:])
        IDF = pool.tile([P, P], FP32)
        make_identity(nc, IDF[:])
        BIASQ = pool.tile([P, K], FP32)
        nc.vector.memset(BIASQ[:, 0:1], 1.0)
        for d in range(2, K + 1):
            nc.vector.memset(BIASQ[:, d - 1:d], float(d * d))

        # ---- masks F_c = (t != c)*BIG, with BIG pad columns ----
        F = pool.tile([P, C, KCH, WP], BF16)
        nc.gpsimd.memset(F[:, :, :, W:WP], BIG)
        for c in range(C):
            nc.vector.tensor_scalar(F[:, c, :, 0:W], T[:], float(c), BIG,
                                    op0=OP.not_equal, op1=OP.mult)

        # ---- row pass: flattened L1 scans, then min ----
        FW = pool.tile([P, C, KCH, WP], BF16)
        BW = pool.tile([P, C, KCH, WP], BF16)
        ff = F[:].rearrange("p c k w -> p (c k w)")
        fwf = FW[:].rearrange("p c k w -> p (c k w)")
        bwf = BW[:].rearrange("p c k w -> p (c k w)")
        HFLAT = 2 * KCH * WP
        ONESB = ONES[:, 0:1].broadcast_to([P, HFLAT])
        for h in range(2):
            lo, hi = h * HFLAT, (h + 1) * HFLAT
            nc.vector.tensor_tensor_scan(fwf[:, lo:hi], ONESB, ff[:, lo:hi],
                                         BIG, op0=OP.add, op1=OP.min)
            nc.vector.tensor_tensor_scan(bwf[:, lo:hi][:, ::-1], ONESB,
                                         ff[:, lo:hi][:, ::-1], BIG,
                                         op0=OP.add, op1=OP.min)
            nc.vector.tensor_tensor(FW[:, 2 * h:2 * h + 2, :, 0:W],
                                    FW[:, 2 * h:2 * h + 2, :, 0:W],
                                    BW[:, 2 * h:2 * h + 2, :, 0:W],
                                    op=OP.min)  # rho

        # ---- PE transposes + fused evictions ----
        # logits first (PE is idle while scans run; E feeds the mid chains),
        # then rho per class (feeds the col pass).
        ET4 = pool.tile([P, C, KW, H], BF16)   # e^logit, transposed
        D1T = pool.tile([P, C, KW, H], BF16)   # rho^2, transposed
        for c in range(C):
            for kw in range(KW):
                pl = psl.tile([P, KCH, P], FP32, tag="psl")
                for kh in range(KCH):
                    nc.tensor.matmul(pl[:, kh, :],
                                     L[:, c, kh, kw * P:(kw + 1) * P],
                                     IDF[:], is_transpose=True)
                nc.scalar.activation(
                    ET4[:, c, kw], pl[:].rearrange("p kh x -> p (kh x)"),
                    ACT.Exp)
        # ---- S_e and 1/S_e on DVE (fills the post-scan gap; 1/S_e folds
        # into the two final fused accumulations) ----
        SE = pool.tile([P, KW, H], BF16)
        TMP = pool.tile([P, KW, H], BF16)
        nc.vector.tensor_tensor(SE[:], ET4[:, 0], ET4[:, 1], op=OP.add)
        nc.vector.tensor_tensor(TMP[:], ET4[:, 2], ET4[:, 3], op=OP.add)
        nc.vector.tensor_tensor(SE[:], SE[:], TMP[:], op=OP.add)
        RC = pool.tile([P, KW, H], FP32)
        nc.vector.reciprocal(RC[:], SE[:])

        # ---- col pass, split into two independent 2-class chains so the
        # first half's sqrt/stats/products start while the second half runs.
        # TMPA_d = d1 + d^2: d=1 halves on DVE at 4x, d>=2 halves on ACT.
        IND = pool.tile([P, C, KW, H], BF16)
        IE = pool.tile([P, C, KW, H], BF16)
        ETP = pool.tile([P, KW, H], BF16)      # e_t (raw)
        ET2 = pool.tile([P, KW, H], BF16)
        TMPA = {}
        for d in range(1, K + 1):
            tmpa_d = pool.tile([P, C, KW, H], BF16, name=f"tmpa{d}")
            TMPA[d] = tmpa_d
        CUR = pool.tile([P, C, KW, H], BF16)
        R = pool.tile([P, C, KW, H], BF16)

        for half in range(2):
            cs = slice(2 * half, 2 * half + 2)
            for c in range(2 * half, 2 * half + 2):
                p9 = psq.tile([P, KW, KCH, P], BF16, tag="ps9")
                for kw in range(KW):
                    for kh in range(KCH):
                        nc.tensor.matmul(p9[:, kw, kh, :],
                                         FW[:, c, kh, kw * P:(kw + 1) * P],
                                         IDENT[:], is_transpose=True)
                nc.scalar.activation(
                    D1T[:, c], p9[:].rearrange("p kw kh x -> p kw (kh x)"),
                    ACT.Square)
            # indicators for the e_t chain (Pool picks these up right away)
            nc.vector.tensor_scalar(IND[:, cs], D1T[:, cs], 0.0, None,
                                    op0=OP.is_equal)
            for c in range(2 * half, 2 * half + 2):
                nc.gpsimd.tensor_tensor(IE[:, c], IND[:, c], ET4[:, c],
                                        op=OP.mult)
            # TMPA builds: ACT for d=1,2; DVE 4x tensor_scalar for d>=3
            nc.scalar.activation(TMPA[1][:, cs], D1T[:, cs], ACT.Identity,
                                 bias=BIASQ[:, 0:1], scale=1.0)
            for d in range(2, K + 1):
                if d == 2:
                    nc.scalar.activation(TMPA[d][:, cs], D1T[:, cs],
                                         ACT.Identity,
                                         bias=BIASQ[:, d - 1:d], scale=1.0)
                else:
                    nc.vector.tensor_scalar(TMPA[d][:, cs], D1T[:, cs],
                                            float(d * d), None, op0=OP.add)
            nc.vector.tensor_scalar(CUR[:, cs, :, H - 1:H],
                                    D1T[:, cs, :, H - 1:H], 0.0, None,
                                    op0=OP.add)
            nc.vector.tensor_tensor(CUR[:, cs, :, 0:H - 1],
                                    D1T[:, cs, :, 0:H - 1],
                                    TMPA[1][:, cs, :, 1:H], op=OP.min)
            nc.vector.tensor_tensor(CUR[:, cs, :, 1:H], CUR[:, cs, :, 1:H],
                                    TMPA[1][:, cs, :, 0:H - 1], op=OP.min)
            for d in range(2, K + 1):
                nc.vector.tensor_tensor(
                    CUR[:, cs, :, 0:H - d], CUR[:, cs, :, 0:H - d],
                    TMPA[d][:, cs, :, d:H], op=OP.min)
                nc.vector.tensor_tensor(
                    CUR[:, cs, :, d:H], CUR[:, cs, :, d:H],
                    TMPA[d][:, cs, :, 0:H - d], op=OP.min)
            nc.scalar.activation(R[:, cs], CUR[:, cs], ACT.Sqrt)

        # e_t chain tail on Pool (IE parts land while the col chains run)
        nc.gpsimd.tensor_tensor(ETP[:], IE[:, 0], IE[:, 1], op=OP.add)
        nc.gpsimd.tensor_tensor(ET2[:], IE[:, 2], IE[:, 3], op=OP.add)
        nc.gpsimd.tensor_tensor(ETP[:], ETP[:], ET2[:], op=OP.add)

        # ---- products for S = sum_c e_c R_c (second half on Pool) ----
        G0 = pool.tile([P, KW, H], BF16)
        G1 = pool.tile([P, KW, H], BF16)
        G2 = pool.tile([P, KW, H], BF16)
        G3 = pool.tile([P, KW, H], BF16)
        XM = pool.tile([P, KW, H], BF16)
        nc.vector.tensor_tensor(G0[:], ET4[:, 0], R[:, 0], op=OP.mult)
        nc.vector.tensor_tensor(G1[:], ET4[:, 1], R[:, 1], op=OP.mult)
        nc.vector.tensor_tensor(G0[:], G0[:], G1[:], op=OP.add)
        nc.vector.tensor_tensor(G2[:], ET4[:, 2], R[:, 2], op=OP.mult)
        nc.vector.tensor_tensor(G3[:], ET4[:, 3], R[:, 3], op=OP.mult)
        nc.vector.tensor_tensor(G2[:], G2[:], G3[:], op=OP.add)

        # ---- order stats on D (no sqrt wait): m2^2, cert = max D ----
        A2 = pool.tile([P, KW, H], BF16)
        B2 = pool.tile([P, KW, H], BF16)
        C2 = pool.tile([P, KW, H], BF16)
        D2 = pool.tile([P, KW, H], BF16)
        M2D = pool.tile([P, KW, H], BF16)
        M2 = pool.tile([P, KW, H], BF16)
        OUT = pool.tile([P, 4], FP32)
        nc.vector.tensor_tensor(A2[:], CUR[:, 0], CUR[:, 1], op=OP.min)
        nc.vector.tensor_tensor(B2[:], CUR[:, 0], CUR[:, 1], op=OP.max)
        nc.vector.tensor_tensor(C2[:], CUR[:, 2], CUR[:, 3], op=OP.min)
        nc.vector.tensor_tensor(D2[:], CUR[:, 2], CUR[:, 3], op=OP.max)
        nc.vector.tensor_tensor(M2D[:], A2[:], C2[:], op=OP.max)
        nc.vector.tensor_tensor(C2[:], B2[:], D2[:], op=OP.min)
        nc.vector.tensor_tensor(B2[:], B2[:], D2[:], op=OP.max)  # max D
        nc.vector.tensor_tensor(M2D[:], M2D[:], C2[:], op=OP.min)  # 2nd-min
        nc.vector.tensor_reduce(OUT[:, 1:2], B2[:], axis=mybir.AxisListType.XY,
                                op=OP.max)
        nc.scalar.activation(M2[:], M2D[:], ACT.Sqrt)

        # ---- accumulate S/S_e and e_t*m2/S_e ----
        nc.vector.tensor_tensor(XM[:], ETP[:], M2[:], op=OP.mult)
        nc.vector.tensor_tensor(G0[:], G0[:], G2[:], op=OP.add)
        JUNK = G1
        nc.vector.scalar_tensor_tensor(JUNK[:], G0[:], 1.0, RC[:],
                                       op0=OP.mult, op1=OP.mult,
                                       accum_out=OUT[:, 0:1])
        nc.vector.scalar_tensor_tensor(JUNK[:], XM[:], 1.0, RC[:],
                                       op0=OP.mult, op1=OP.mult,
                                       accum_out=OUT[:, 2:3])
        nc.vector.memset(OUT[:, 3:4], 0.0)
        nc.sync.dma_start(out_d[:], OUT[:])

    nc.finalize()
    return nc


_NC_CACHE: dict[int, bass.Bass] = {}


def _get_nc(K: int) -> bass.Bass:
    if K not in _NC_CACHE:
        _NC_CACHE[K] = _build_nc(K)
    return _NC_CACHE[K]


def _run_device(logits: np.ndarray, targets: np.ndarray, K: int, **kw):
    nc = _get_nc(K)
    in_maps = [
        {"logits": np.ascontiguousarray(logits[b], dtype=np.float32),
         "targets": np.ascontiguousarray(targets[b], dtype=np.int32)}
        for b in range(N_CORES)
    ]
    return run_bass_kernel_spmd(nc, in_maps, list(range(N_CORES)), **kw)


# ---------------------------------------------------------------------------
# exact host fallback (degenerate masks / failed certificate; ~never taken)
# ---------------------------------------------------------------------------

def _edt2_exact_np(mask: np.ndarray) -> np.ndarray:
    Hh, Ww = mask.shape
    f = np.where(mask, 0.0, 1e8)
    iw = np.arange(Ww, dtype=np.float64)
    sqw = (iw[:, None] - iw[None, :]) ** 2
    d1 = (f[:, None, :] + sqw[None, :, :]).min(axis=-1)
    ih = np.arange(Hh, dtype=np.float64)
    sqh = (ih[:, None] - ih[None, :]) ** 2
    d2 = (d1[None, :, :] + sqh[:, :, None]).min(axis=1)
    return d2


def _loss_host_exact(logits: np.ndarray, targets: np.ndarray) -> np.float32:
    B = logits.shape[0]
    lo = logits.astype(np.float64)
    mx = lo.max(axis=1, keepdims=True)
    e = np.exp(lo - mx)
    probs = e / e.sum(axis=1, keepdims=True)
    total = 0.0
    for b in range(B):
        for c in range(C):
            m = targets[b] == c
            s = int(m.sum())
            pos = np.sqrt(_edt2_exact_np(m))
            if s == 0:
                phi = pos
            elif s == m.size:
                phi = -np.sqrt(_edt2_exact_np(~m))
            else:
                phi = pos - np.sqrt(_edt2_exact_np(~m)) + 1.0
            total += float((probs[b, c] * phi).sum())
    return np.float32(total / (B * C * H * W))


def kernel(logits: np.ndarray, targets: np.ndarray) -> np.ndarray:
    logits = np.asarray(logits)
    targets = np.asarray(targets)
    assert logits.shape == (N_CORES, C, H, W) and targets.shape == (N_CORES, H, W)

    counts = np.stack([(targets == c).sum(axis=(1, 2)) for c in range(C)])
    if counts.min() == 0 or counts.max() == H * W:
        return np.asarray(_loss_host_exact(logits, targets))

    K = DEFAULT_K
    for _attempt in range(3):
        res = _run_device(logits, targets, K).results
        out = np.stack([res[b]["out"] for b in range(N_CORES)])  # (8, 128, 4)
        maxd = float(out[:, :, 1].max())
        # cert: every computed D <= (K+1)^2 is provably exact
        if maxd <= (K + 1) ** 2 + 0.5:
            total = (float(out[:, :, 0].astype(np.float64).sum())
                     - float(out[:, :, 2].astype(np.float64).sum()))
            return np.asarray(
                np.float32(total / (N_CORES * C * H * W) + 1.0 / C))
        if maxd > 4000.0 ** 2:  # sentinel leaked: a window saw no features
            break
        K = int(np.ceil(np.sqrt(maxd)))
    return np.asarray(_loss_host_exact(logits, targets))
